# revision 18
# baseline (speedup 1.0000x reference)
"""Distributed Trainium2 kernel for single-head attention with QKV projections.

Reference computation (B=4, N=4096, D=256, fp32):
    q = x @ Wq_w.T + Wq_b
    k = z @ Wk_w.T + Wk_b
    v = z @ Wv_w.T + Wv_b
    out = softmax(q @ k.T / sqrt(D)) @ v

Sharding: pure data-parallel over (batch, query-half) across 8 cores. Core c
handles batch b = c//2, query rows [h*2048, (h+1)*2048) with h = c%2, and
holds the full z[b] so K/V are recomputed per core. No collectives.

Algebra: q.k = x Wq^T Wk z^T + (bq Wk).z + const(query), and the
query-constant terms cancel in softmax. With M = Wq^T Wk folded on host:
    scores = (x M + 1.(bq Wk)) z^T
so the K projection disappears and the per-key score bias rides as a
per-partition bias on the qM-projection copy, not as an extra matmul column.

fp8 scores + first-order error correction: the N^2 D scores matmul runs
entirely in fp8e4 (e4m3) with perf_mode=DoubleRow — both 128-feature chunks
packed into one virtual 256-row matmul at ~1.8x bf16 tile throughput. Raw
fp8 costs ~2.0e-2 rel-err, dominated by the qM-side quantization noise
dq = qMb - qM8, whose effect on the output is COHERENT across keys:
    delta_out[i] ~= -(dq[:,i]^T Wv) / 16
(because sum_j w_j z_j (v_j - out)^T ~= E[z z^T] Wv ~= Wv for whitened z).
The kernel computes this correction with a cheap [256 x 2048 x 256] bf16
matmul on dq and folds it into the output bias:  bc = Wv_b + (dq^T Wv)/16,
recovering rel-err ~1.24e-2. The z-side noise is incoherent and stays.
PV always stays bf16 (its DoubleRow variant is weight-load-bound at FD=257).

Layout: everything is kept "transposed" so no PE transposes are needed:
  - qM[dz, i] leaves the projection with features on partitions, exactly the
    scores-rhs layout (bf16 master + fp8 copy, both via ACT); zT8 is the lhsT.
  - scores are computed transposed, sT[j, i] (keys on partitions), so exp'd
    probabilities are directly the lhsT of the PV matmul.
  - a ones-column appended to v makes the PV matmul also produce the softmax
    denominator; normalization + bias-add fuse into one DVE op per sq block.

exp runs on ACT in [128, 1024] instructions (a 2-bank PSUM tile holding two
key-tiles' scores) — the ~370ns fixed access latency per ACT instruction
amortizes over 1024 columns. Possible because the bias fold removed the
per-key-tile exp bias. Projection copies are 1024-wide for the same reason.

Scheduling (engine queues are strict FIFO, so emit order IS the schedule):
  PE : warm, proj, vproj, scores(qb0), then qb=1..4 interleaving scores(qb)
       with PV(qb-1) at key-pair granularity; the correction matmuls ride
       inside the qb=1 iteration (PSUM has no free bank for a solo phase).
  ACT: qM copies (bf16+fp8), odd vproj drains, then exp stream.
  DVE: memsets, even vproj drains, dq, bc, then normalize/output stream.
vproj drains split across ACT/DVE because one engine alone (~0.5us per
[128,256] PSUM read) cannot keep pace with the PE's vproj matmuls.
PV consumes exp'd tiles in production order; the last 4 tiles run sq-major
so per-sq accumulations finish staggered and normalize+DMA overlap the
remaining matmuls. Output DMA is one batched [128, 4, 256] descriptor set
per query block, except the last block which goes per-sq to cut the tail.

A warm-up accumulation group (back-to-back matmuls into one PSUM bank)
bridges the input-DMA window because the PE clock ramps 0.65 -> 2.4 GHz over
~3us of continuous execution and resets on idle.
"""

import numpy as np
import ml_dtypes

B, N, D = 4, 4096, 256
NCORES = 8
S = N // 2          # query rows per core
P = 128             # partitions
QBLK = 512          # scores free-dim block (one PSUM bank)
NJT = N // P        # 32 key tiles
NPAIR = NJT // 2    # 16 key-tile pairs
NQB = S // QBLK     # 4 query blocks per core
NBLK = S // P       # 16 query sub-blocks (drain/correction granularity)
DC = D // P         # 2 chunks of the feature dim
NWARM = 3           # PE p-state warm-up matmuls (bridge until first input DMA)
VW = D + 1          # v tile width: [v | 1]

BF16 = ml_dtypes.bfloat16
FP8 = ml_dtypes.float8_e4m3

_CACHE = {}


def _build():
    import concourse.mybir as mybir
    import concourse.tile as tile
    from concourse import bacc

    bf16 = mybir.dt.bfloat16
    fp8 = mybir.dt.float8e4
    f32 = mybir.dt.float32
    AF = mybir.ActivationFunctionType
    ALU = mybir.AluOpType
    DR = mybir.MatmulPerfMode.DoubleRow

    nc = bacc.Bacc("TRN2", target_bir_lowering=False, debug=False, num_devices=NCORES)

    # xT: [128, 2*S] = [chunk0 | chunk1] bf16 (proj rhs)
    # zT: [128, 2*N] bf16 = [c0h0|c1h0|c0h1|c1h1] (v-proj lhsT)
    # zT8: [128, NJT*256] fp8, tile-major: [p, t*256 + c*128 + j] (scores lhsT)
    # Wpack cols: [M chunk0 | M chunk1 | WvT chunk0 | WvT chunk1] bf16
    # Bpack: [128, D+2] f32 = [Wv_b broadcast | bqWk chunk0 | bqWk chunk1]
    xT = nc.dram_tensor("xT", [P, DC * S], bf16, kind="ExternalInput").ap()
    zT = nc.dram_tensor("zT", [P, DC * N], bf16, kind="ExternalInput").ap()
    zT8 = nc.dram_tensor("zT8", [P, NJT * D], fp8, kind="ExternalInput").ap()
    Wpack = nc.dram_tensor("Wpack", [P, 4 * D], bf16, kind="ExternalInput").ap()
    Bpack = nc.dram_tensor("Bpack", [P, D + 2], f32, kind="ExternalInput").ap()
    out = nc.dram_tensor("out", [S, D], f32, kind="ExternalOutput").ap()

    with tile.TileContext(nc) as tc:
        with (
            tc.tile_pool(name="consts", bufs=1) as cp,
            tc.tile_pool(name="big", bufs=1) as bp,
            tc.tile_pool(name="pblk", bufs=2) as pp,
            tc.tile_pool(name="outp", bufs=8) as op,
            tc.tile_pool(name="psum", bufs=1, space="PSUM") as ps,
        ):
            # ---- PE warm-up: one accumulation group, back-to-back ----
            wrm = cp.tile([P, P + QBLK], bf16, tag="warm", name="warm")
            nc.vector.memset(wrm[:], 0.0)
            wps = ps.tile([P, 2 * QBLK], f32, tag="sc", bufs=2, name="warm_ps")
            for i in range(NWARM):
                nc.tensor.matmul(
                    wps[:, 0:QBLK], wrm[:, 0:P], wrm[:, P:P + QBLK],
                    start=(i == 0), stop=(i == NWARM - 1),
                )

            # ---- input / constant DMAs (ordered by first use) ----
            # DMA order = need order; the 16 DMA engines fair-share active
            # transfers, so later queue positions effectively deprioritize.
            # xT is split per (jp, chunk) so proj block jp0 starts on 0.5MB
            # instead of the full 1MB; z halves likewise stage in.
            wpk = cp.tile([P, 4 * D], bf16, tag="wpk", name="wpk")
            nc.sync.dma_start(wpk[:], Wpack[:])
            xTp = bp.tile([P, DC * S], bf16, tag="xTp", name="xTp")
            for c in range(DC):
                lo = c * S
                nc.sync.dma_start(
                    xTp[:, lo:lo + 2 * QBLK], xT[:, lo:lo + 2 * QBLK]
                )
            bpk = cp.tile([P, D + 2], f32, tag="bpk", name="bpk")
            nc.sync.dma_start(bpk[:], Bpack[:])
            zT8p = bp.tile([P, NJT * D], fp8, tag="zT8p", name="zT8p")
            zTp = bp.tile([P, DC * N], bf16, tag="zTp", name="zTp")
            nc.sync.dma_start(zT8p[:, 0:NJT * D // 2], zT8[:, 0:NJT * D // 2])
            for c in range(DC):
                lo = c * S + 2 * QBLK
                nc.sync.dma_start(
                    xTp[:, lo:lo + 2 * QBLK], xT[:, lo:lo + 2 * QBLK]
                )
            for hh in range(4):  # zT in quarters: vproj tiles stage in
                nc.sync.dma_start(
                    zTp[:, hh * (N // 2):(hh + 1) * (N // 2)],
                    zT[:, hh * (N // 2):(hh + 1) * (N // 2)],
                )
            nc.sync.dma_start(
                zT8p[:, NJT * D // 2:NJT * D], zT8[:, NJT * D // 2:NJT * D]
            )

            def xs(c, lo, hi):   # xT chunk c, query cols [lo, hi)
                return xTp[:, c * S + lo:c * S + hi]

            def zs(c, lo, hi):   # zT chunk c, key cols [lo, hi) (within a half)
                h, r = divmod(lo, N // 2)
                assert hi - lo <= N // 2 - r
                o = h * N + c * (N // 2) + r
                return zTp[:, o:o + (hi - lo)]

            def zs8(t_i):        # fp8 scores lhsT for key tile t_i: [128, 2, 128]
                return zT8p[:, t_i * D:(t_i + 1) * D].rearrange(
                    "p (c j) -> p c j", c=DC
                )

            def m_sl(c):   # M lhsT chunk c: [128, 256]
                return wpk[:, c * D:(c + 1) * D]

            def wv_sl(c):  # WvT rhs chunk c: [128, 256]
                return wpk[:, (DC + c) * D:(DC + c + 1) * D]

            bvb_sb = bpk[:, 0:D]

            # ---- qM projection: (x M + bqWk)^T[dz, i] over [256, 2048] ----
            # bf16 master (ACT Identity + per-chunk bias), fp8 copy (ACT cast).
            qMb = bp.tile([P, DC * S], bf16, tag="qMb", name="qMb")
            qM8 = bp.tile([P, DC * S], fp8, tag="qM8", name="qM8")
            qM8r = qM8[:].rearrange("p (c i) -> p c i", c=DC)
            for jp in range(S // (2 * QBLK)):
                for e in range(DC):
                    acc = ps.tile([P, 2 * QBLK], f32, tag="sc", bufs=2,
                                  name="proj_ps")
                    for half in range(2):
                        jb = 2 * jp + half
                        for c in range(DC):
                            nc.tensor.matmul(
                                acc[:, half * QBLK:(half + 1) * QBLK],
                                m_sl(c)[:, e * P:(e + 1) * P],
                                xs(c, jb * QBLK, (jb + 1) * QBLK),
                                start=(c == 0),
                                stop=(c == DC - 1),
                            )
                    lo = e * S + 2 * jp * QBLK
                    nc.scalar.activation(
                        qMb[:, lo:lo + 2 * QBLK], acc[:], AF.Identity,
                        bias=bpk[:, D + e:D + e + 1],
                    )
                    # fp8 cast on DVE from the bf16 master — keeps ACT free
                    # for the exp stream and avoids a same-bank PSUM co-read
                    nc.vector.tensor_copy(
                        qM8[:, lo:lo + 2 * QBLK], qMb[:, lo:lo + 2 * QBLK]
                    )

            # ---- v projection: [v | 1] per key tile, bf16 ----
            # Emitted inside the attention pipeline (early slack); 2 key
            # tiles share one PSUM bank (one accumulation group, halves
            # written in the has_written-friendly order), DVE drains.
            vbig = bp.tile([P, NJT * VW], bf16, tag="vbig", name="vbig")
            nc.vector.memset(
                vbig[:].rearrange("p (t w) -> p t w", w=VW)[:, :, D:D + 1], 1.0
            )

            def emit_vproj_sg(sg):
                # sc tag (transient rotation with scores/corr/proj tiles) —
                # the pv tag holds the long-lived PV accumulators and a
                # vproj alloc there would deadlock on a later-emitted drain
                acc = ps.tile([P, 2 * D], f32, tag="sc", bufs=2, name="v_ps")
                first = True
                for ti in range(2):
                    t_i = 2 * sg + ti
                    for c in range(DC):
                        nc.tensor.matmul(
                            acc[:, ti * D:(ti + 1) * D],
                            zs(c, t_i * P, (t_i + 1) * P),
                            wv_sl(c),
                            start=first,
                            stop=(ti == 1 and c == DC - 1),
                        )
                        first = False
                dst = vbig[:, 2 * sg * VW:(2 * sg + 2) * VW].rearrange(
                    "p (t w) -> p t w", w=VW
                )[:, :, 0:D]
                src = acc[:].rearrange("p (t w) -> p t w", w=D)
                nc.vector.tensor_copy(dst, src)

            def v_sl(t_i):
                return vbig[:, t_i * VW:t_i * VW + D + 1]

            # dq = qMb - qM8 (bf16), for the first-order fp8 correction;
            # emitted inside the pipeline, 2 blocks per slot
            dq = bp.tile([P, DC * S], bf16, tag="dq", name="dq")

            def emit_dq(jp):
                for e in range(DC):
                    lo = e * S + 2 * jp * QBLK
                    nc.vector.scalar_tensor_tensor(
                        dq[:, lo:lo + 2 * QBLK], qM8[:, lo:lo + 2 * QBLK],
                        -1.0, qMb[:, lo:lo + 2 * QBLK],
                        op0=ALU.mult, op1=ALU.add,
                    )
            # bc[blk] = Wv_b + (dq[:, blk]^T WvT)/16, per 128-query block;
            # tail cols hold Wv_b tiled x4, built on device (saves DMA bytes)
            bc = bp.tile([P, (NBLK + 4) * D], f32, tag="bc", name="bc")
            for k in range(4):
                nc.vector.tensor_copy(
                    bc[:, (NBLK + k) * D:(NBLK + k + 1) * D], bvb_sb
                )
            bvb4_sb = bc[:, NBLK * D:(NBLK + 4) * D]

            def emit_corr_group(g):
                # 4 query blocks -> one [128, 1024] PSUM tile + one DVE stt
                cacc = ps.tile([P, 2 * QBLK], f32, tag="sc", bufs=2,
                               name="corr_ps")
                for b in range(4):
                    blk = 4 * g + b
                    for c in range(DC):
                        nc.tensor.matmul(
                            cacc[:, b * D:(b + 1) * D],
                            dq[:, c * S + blk * P:c * S + (blk + 1) * P],
                            wv_sl(c),
                            start=(c == 0), stop=(c == DC - 1),
                        )
                nc.vector.scalar_tensor_tensor(
                    bc[:, g * 4 * D:(g + 1) * 4 * D], cacc[:],
                    1.0 / 16.0, bvb4_sb, op0=ALU.mult, op1=ALU.add,
                )

            # ---- attention, software-pipelined: scores(qb) || PV(qb-1) ----
            ptbs = [None, None]

            def emit_scores_pair(qb, u):
                st = ps.tile([P, 2 * QBLK], f32, tag="sc", bufs=2, name="sc_ps")
                for k in range(2):
                    nc.tensor.matmul(
                        st[:, k * QBLK:(k + 1) * QBLK],
                        zs8(2 * u + k),
                        qM8r[:, :, qb * QBLK:(qb + 1) * QBLK],
                        start=True, stop=True, perf_mode=DR,
                    )
                ptb = ptbs[qb % 2]
                nc.scalar.activation(
                    ptb[:, 2 * u * QBLK:(2 * u + 2) * QBLK], st[:],
                    AF.Exp, scale=1.0 / 16.0,
                )

            def emit_pv(pb, pvs, t_i, sq):
                ptb = ptbs[pb % 2]
                o = t_i * QBLK + sq * P
                nc.tensor.matmul(
                    pvs[sq][:], ptb[:, o:o + P], v_sl(t_i),
                    start=(t_i == 0), stop=(t_i == NJT - 1),
                )

            def emit_drain(pb, pvs, sq, ob):
                pv = pvs[sq]
                gblk = pb * 4 + sq
                recip = ob[:, 4 * D + sq:4 * D + sq + 1]
                nc.vector.reciprocal(recip, pv[:, D:D + 1])
                nc.vector.scalar_tensor_tensor(
                    ob[:, sq * D:(sq + 1) * D], pv[:, 0:D], recip,
                    bc[:, gblk * D:(gblk + 1) * D],
                    op0=ALU.mult, op1=ALU.add,
                )
                if pb == NQB - 1:  # last block: per-sq DMA to shorten the tail
                    r0 = gblk * P
                    nc.sync.dma_start(
                        out[r0:r0 + P, :], ob[:, sq * D:(sq + 1) * D]
                    )

            # ---- unified pipeline over one global stream ----
            # Score-pairs flow in order (qb, u); a PV cursor (tile-sq units,
            # 128 per block) chases them with a 2-pair lag; vproj, dq, and
            # the correction matmuls ride in fixed early slots; per-block
            # drains + output DMA fire as the cursor crosses block tails.
            LAG = 2
            NU = 128          # pv units per block (32 tiles x 4 sq)
            pvs_by_pb = {}
            state = {"cursor": 0}
            # slot layout keeps the DVE FIFO causal: dq(jp0) before the
            # correction groups it feeds, dq(jp1) before groups 2-3, and
            # bc(0) lands before block 0's drains (cursor crosses at g=15)
            dq_slots = {9: 0, 16: 1}
            corr_slots = {11: 0, 13: 1, 18: 2, 20: 3}

            def emit_block_tail(pb):
                pvs = pvs_by_pb[pb]
                ob = op.tile([P, 4 * D + 4], f32, tag="ob", bufs=2, name="ob")
                for sq in range(QBLK // P):
                    for t_i in range(NJT - 4, NJT):
                        emit_pv(pb, pvs, t_i, sq)
                    emit_drain(pb, pvs, sq, ob)
                if pb < NQB - 1:
                    dst = out[pb * 4 * P:(pb + 1) * 4 * P, :].rearrange(
                        "(s p) e -> p s e", p=P
                    )
                    nc.sync.dma_start(
                        dst, ob[:, 0:4 * D].rearrange("p (s e) -> p s e", s=4)
                    )

            def advance_pv(target):
                while state["cursor"] < min(target, NQB * NU):
                    pb, r = divmod(state["cursor"], NU)
                    if r == 0:
                        pvs_by_pb[pb] = [
                            ps.tile([P, D + 1], f32, tag="pv", bufs=4,
                                    name=f"pv_ps{sq}")
                            for sq in range(QBLK // P)
                        ]
                    if r == NU - 16:  # last 4 tiles sq-major, staggered
                        emit_block_tail(pb)
                        state["cursor"] += 16
                        continue
                    t_i, sq = divmod(r, 4)
                    emit_pv(pb, pvs_by_pb[pb], t_i, sq)
                    state["cursor"] += 1

            for g in range(NQB * NPAIR):
                qb, u = divmod(g, NPAIR)
                if u == 0:
                    ptbs[qb % 2] = pp.tile(
                        [P, NJT * QBLK], bf16, tag="pT", name="pT"
                    )
                emit_scores_pair(qb, u)
                if g < NPAIR:
                    emit_vproj_sg(g)
                if g in dq_slots:
                    emit_dq(dq_slots[g])
                if g in corr_slots:
                    emit_corr_group(corr_slots[g])
                advance_pv(8 * (g - LAG + 1))
            advance_pv(NQB * NU)

    nc.compile()
    return nc


def _get_nc():
    if "nc" not in _CACHE:
        _CACHE["nc"] = _build()
    return _CACHE["nc"]


def _prep_in_maps(x, z, Wq_w, Wq_b, Wk_w, Wk_b, Wv_w, Wv_b):
    x = np.asarray(x, np.float32)
    z = np.asarray(z, np.float32)
    Wq = np.asarray(Wq_w, np.float64)
    Wk = np.asarray(Wk_w, np.float64)
    bq = np.asarray(Wq_b, np.float64)

    M = (Wq.T @ Wk).astype(np.float32)           # [dx, dz]
    bqWk = (bq @ Wk).astype(np.float32)          # [dz]
    WvT = np.ascontiguousarray(np.asarray(Wv_w, np.float32).T)  # [dz, e]

    Wpack = np.empty((P, 4 * D), BF16)
    for c in range(DC):
        Wpack[:, c * D:(c + 1) * D] = M[c * P:(c + 1) * P, :].astype(BF16)
        Wpack[:, (DC + c) * D:(DC + c + 1) * D] = WvT[c * P:(c + 1) * P, :].astype(BF16)
    Bpack = np.empty((P, D + 2), np.float32)
    Bpack[:, 0:D] = np.broadcast_to(np.asarray(Wv_b, np.float32), (P, D))
    for c in range(DC):
        Bpack[:, D + c] = bqWk[c * P:(c + 1) * P]

    in_maps = []
    for core in range(NCORES):
        b, h = divmod(core, 2)
        xTc = np.ascontiguousarray(x[b].T[:, h * S:(h + 1) * S]).astype(BF16)
        xTp = np.hstack([xTc[0:P], xTc[P:2 * P]])
        zTc = np.ascontiguousarray(z[b].T)
        zTb = zTc.astype(BF16)
        zTp = np.hstack([
            zTb[0:P, 0:N // 2], zTb[P:2 * P, 0:N // 2],
            zTb[0:P, N // 2:N], zTb[P:2 * P, N // 2:N],
        ])
        # fp8 z, tile-major: [p, t*256 + c*128 + j] = z[t*128+j, c*128+p]
        z8 = zTc.astype(FP8).reshape(DC, P, NJT, P)        # [c, p, t, j]
        zT8p = np.ascontiguousarray(
            z8.transpose(1, 2, 0, 3).reshape(P, NJT * D)
        )
        in_maps.append({
            "xT": xTp, "zT": zTp, "zT8": zT8p,
            "Wpack": Wpack, "Bpack": Bpack,
        })
    return in_maps


def kernel(x, z, Wq_w, Wq_b, Wk_w, Wk_b, Wv_w, Wv_b):
    from concourse.bass_utils import run_bass_kernel_spmd

    in_maps = _prep_in_maps(x, z, Wq_w, Wq_b, Wk_w, Wk_b, Wv_w, Wv_b)
    nc = _get_nc()
    _CACHE["in_maps"] = in_maps
    res = run_bass_kernel_spmd(nc, in_maps, core_ids=list(range(NCORES)))

    full = np.empty((B, N, D), np.float32)
    for core in range(NCORES):
        b, h = divmod(core, 2)
        full[b, h * S:(h + 1) * S, :] = res.results[core]["out"]
    return full
